# revision 23
# baseline (speedup 1.0000x reference)
"""Causal self-attention (B=4, T=2048, C=1024, 16 heads) on 8 NeuronCores.

Sharding: core c -> batch b=c//2, head group g=c%2 (8 heads each).
Each core computes q,k,v for its 8 heads, causal attention, and a partial
output projection (row-slice of w_proj). Host sums the two partials per
batch and adds b_proj.

v3 design:
  - all matmul operands bf16 (1 cycle/row on PE; fp32r ran fp32_mode=HIGH
    at 2-4 cycles/row).
  - S^T tiles [k,q] via row-tiled head pairs (K=64 stationaries at base
    partitions 0/64 run concurrently on the two PE row halves).
  - triangular restriction: diagonal k-tiles only compute q >= k; causal
    mask multiply only on the leading 128-wide triangle block.
  - PV in natural layout: Y[q,hd] = px_slice^T @ v_aug, px [k,128q] as
    stationary, v plus a ones column (-> softmax denominator at col 64)
    as the 65-wide moving operand. 4 (qs,h) accumulators share a PSUM
    bank; only the bank's first matmul uses start=True (start clears
    has_written for the WHOLE bank).
  - normalization: DVE copies each yacc bank to SBUF, then GPSIMD
    normalize_recip does y/denom per partition (keeps the DVE free for
    mask multiplies).
  - y_nat transposed back per 128-tile on the PE for the projection.
  - software pipelining via a global deadline-tagged filler queue: V
    tiles, QK projection chunks, y transposes and projection groups are
    popped 2 per tk into the PE's exp-wait bubbles; forced drains happen
    only when the next consumer needs them.
  - exp(tk) emitted before S(tk+1) so ACT is never gated by PV/fillers.
  - warmup matmuls keep the PE HAM unthrottled during the initial DMA.
"""
import numpy as np
import ml_dtypes
import concourse.bass as bass
from concourse import bacc
import concourse.tile as tile
import concourse.mybir as mybir
from concourse.bass_utils import run_bass_kernel_spmd

B, T, C = 4, 2048, 1024
HD = 64            # head dim
PAIRS = 4          # local head pairs (8 heads per core)
KT = C // 128      # 8 contraction tiles for qkv
TT = T // 128      # 16 row tiles of T
NQ = T // 512      # 4 query chunks of 512
BF16 = mybir.dt.bfloat16
F32 = mybir.dt.float32
EXP = mybir.ActivationFunctionType.Exp
WARMUP_MM = 90

_NC_CACHE = {}


def _build(bias=False):
    nc = bacc.Bacc("TRN2", target_bir_lowering=False, debug=False)
    xT_d = nc.dram_tensor("xT", [C, T], BF16, kind="ExternalInput")
    wqkv_d = nc.dram_tensor("wqkv", [C, 1536], BF16, kind="ExternalInput")
    bqkv_d = nc.dram_tensor("bqkv", [1536], BF16, kind="ExternalInput")
    wp_d = nc.dram_tensor("wp", [512, C], BF16, kind="ExternalInput")
    out_d = nc.dram_tensor("out", [T, C], F32, kind="ExternalOutput")

    tri_np = np.zeros((128, 128), dtype=np.float32)
    for p in range(128):
        tri_np[p, p:] = 1.0
    tri_d = nc.inline_tensor(tri_np.astype(ml_dtypes.bfloat16), name="tri")
    eye_d = nc.inline_tensor(np.eye(128, dtype=np.float32)
                             .astype(ml_dtypes.bfloat16), name="eye")
    ones_d = nc.inline_tensor(np.ones((1, 128), dtype=np.float32)
                              .astype(ml_dtypes.bfloat16), name="onesr")

    with tile.TileContext(nc) as tc:
        with (
            tc.tile_pool(name="x", bufs=8) as p_x,
            tc.tile_pool(name="w", bufs=8) as p_w,
            tc.tile_pool(name="wp", bufs=4) as p_wp,
            tc.tile_pool(name="va", bufs=16) as p_va,
            tc.tile_pool(name="qk", bufs=4) as p_qk,
            tc.tile_pool(name="px", bufs=5) as p_px,
            tc.tile_pool(name="stg", bufs=4) as p_stg,
            tc.tile_pool(name="yn", bufs=4) as p_yn,
            tc.tile_pool(name="yt", bufs=4) as p_yt,
            tc.tile_pool(name="ob", bufs=2) as p_ob,
            tc.tile_pool(name="tiny", bufs=2) as p_tiny,
            tc.tile_pool(name="st", bufs=2, space="PSUM") as pp_st,
            tc.tile_pool(name="ya", bufs=2, space="PSUM") as pp_y,
            tc.tile_pool(name="mm", bufs=2, space="PSUM") as pp_mm,
        ):
            # ---- constants ----
            tri = p_tiny.tile([128, 128], BF16, tag="tri")
            nc.sync.dma_start(out=tri[:], in_=tri_d.ap())
            eye = p_tiny.tile([128, 128], BF16, tag="eye")
            nc.sync.dma_start(out=eye[:], in_=eye_d.ap())
            wdum = p_tiny.tile([128, 512], BF16, tag="wdum")
            nc.vector.memset(wdum[:], 0.0)

            # warmup: keep PE busy (HAM unthrottled) during input DMA
            warm = pp_st.tile([128, 1024], F32, tag="st", name="warm")
            for i in range(WARMUP_MM):
                nc.tensor.matmul(warm[:, 0:512], eye[:], wdum[:],
                                 start=True, stop=True)

            # ---- input loads ----
            xts = []
            for k in range(KT):
                t_ = p_x.tile([128, T], BF16, tag="xt", name=f"xt{k}")
                nc.sync.dma_start(out=t_[:],
                                  in_=xT_d.ap()[k * 128:(k + 1) * 128, :])
                xts.append(t_)
            wqs = []
            for k in range(KT):
                w = p_w.tile([128, 1536], BF16, tag="wq", name=f"wq{k}")
                nc.sync.dma_start(out=w[:],
                                  in_=wqkv_d.ap()[k * 128:(k + 1) * 128, :])
                wqs.append(w)
            wps = []
            for k in range(PAIRS):
                w = p_wp.tile([128, C], BF16, tag="wp", name=f"wp{k}")
                nc.sync.dma_start(out=w[:],
                                  in_=wp_d.ap()[k * 128:(k + 1) * 128, :])
                wps.append(w)
            if bias:
                ones_r = p_tiny.tile([1, 128], BF16, tag="onesr")
                nc.sync.dma_start(out=ones_r[:], in_=ones_d.ap())
                bv = p_tiny.tile([1, 512], BF16, tag="bv")
                nc.sync.dma_start(out=bv[:],
                                  in_=bqkv_d.ap()[1024:1536].unsqueeze(0))
                bqs, bks = [], []
                for hp in range(PAIRS):
                    bq = p_tiny.tile([128, 1], BF16, tag=f"bq{hp}")
                    nc.sync.dma_start(out=bq[:], in_=bqkv_d.ap()
                                      [hp * 128:(hp + 1) * 128].unsqueeze(1))
                    bqs.append(bq)
                    bk = p_tiny.tile([128, 1], BF16, tag=f"bk{hp}")
                    nc.sync.dma_start(out=bk[:], in_=bqkv_d.ap()
                                      [512 + hp * 128:512 + (hp + 1) * 128]
                                      .unsqueeze(1))
                    bks.append(bk)

            # ---- emit helpers ----
            vaugs = [None] * TT

            def v_tile_ops(t):
                """9 closures producing vaug[t]."""
                st = {}

                def mm(k):
                    if k == 0:
                        st['ps'] = pp_mm.tile([128, 512], F32, tag="mm",
                                              name=f"psv{t}")
                    nc.tensor.matmul(st['ps'][:],
                                     xts[k][:, t * 128:(t + 1) * 128],
                                     wqs[k][:, 1024:1536], start=(k == 0),
                                     stop=(k == KT - 1 and not bias))
                    if k == KT - 1:
                        if bias:
                            nc.tensor.matmul(st['ps'][:], ones_r[:], bv[:],
                                             start=False, stop=True)
                        va = p_va.tile([128, 8 * 65], BF16, tag="va",
                                       name=f"va{t}")
                        nc.vector.tensor_copy(
                            va[:].rearrange("p (l c) -> p l c", c=65)
                            [:, :, 0:64],
                            st['ps'][:].rearrange("p (l c) -> p l c", c=64))
                        nc.gpsimd.memset(
                            va[:].rearrange("p (l c) -> p l c", c=65)
                            [:, :, 64:65], 1.0)
                        vaugs[t] = va
                return [lambda k=k: mm(k) for k in range(KT)]

            qkt = {}

            def qk_chunk_ops(hp, n):
                """16 closures: 8 q-matmuls then 8 k-matmuls for chunk n."""
                sl = slice(n * 512, (n + 1) * 512)
                st = {}

                def emit_q(k):
                    if n == 0 and k == 0:
                        qkt[hp] = (
                            p_qk.tile([128, T], BF16, tag="qk",
                                      name=f"qt{hp}"),
                            p_qk.tile([128, T], BF16, tag="qk",
                                      name=f"kt{hp}"),
                        )
                    if k == 0:
                        st['q'] = pp_mm.tile([128, 512], F32, tag="mm",
                                             name=f"psq{hp}_{n}")
                    nc.tensor.matmul(st['q'][:],
                                     wqs[k][:, hp * 128:(hp + 1) * 128],
                                     xts[k][:, sl], start=(k == 0),
                                     stop=(k == KT - 1))
                    if k == KT - 1:
                        if bias:
                            nc.vector.tensor_scalar_add(
                                qkt[hp][0][:, sl], st['q'][:],
                                bqs[hp][:, 0:1])
                        else:
                            nc.vector.tensor_copy(qkt[hp][0][:, sl],
                                                  st['q'][:])

                def emit_k(k):
                    if k == 0:
                        st['k'] = pp_mm.tile([128, 512], F32, tag="mm",
                                             name=f"psk{hp}_{n}")
                    nc.tensor.matmul(st['k'][:],
                                     wqs[k][:, 512 + hp * 128:
                                             512 + (hp + 1) * 128],
                                     xts[k][:, sl], start=(k == 0),
                                     stop=(k == KT - 1))
                    if k == KT - 1:
                        if bias:
                            nc.vector.tensor_scalar_add(
                                qkt[hp][1][:, sl], st['k'][:],
                                bks[hp][:, 0:1])
                        else:
                            nc.vector.tensor_copy(qkt[hp][1][:, sl],
                                                  st['k'][:])

                return ([lambda k=k: emit_q(k) for k in range(KT)]
                        + [lambda k=k: emit_k(k) for k in range(KT)])

            ynat, ytr = {}, {}

            def t_ops(hp, qb):
                """Transpose 4 t-tiles of pair hp's natural y for qb."""
                st = {}

                def emit(i):
                    if i == 0:
                        st['tp'] = pp_mm.tile([128, 512], BF16, tag="mm",
                                              name=f"tp{hp}_{qb}")
                    t = qb * 4 + i
                    nc.tensor.transpose(st['tp'][:, i * 128:(i + 1) * 128],
                                        ynat[hp][:, t * 128:(t + 1) * 128],
                                        eye[:])
                    nc.vector.tensor_copy(
                        ytr[hp][:, t * 128:(t + 1) * 128],
                        st['tp'][:, i * 128:(i + 1) * 128])
                return [lambda i=i: emit(i) for i in range(4)]

            def proj_ops(t, scalar_copy=False):
                """8 single-matmul closures for out t-tile + copy + DMA."""
                st = {}

                def emit(cc, k):
                    csl = slice(cc * 512, (cc + 1) * 512)
                    if cc == 0 and k == 0:
                        st['ob'] = p_ob.tile([128, 1024], F32, tag="ob",
                                             name=f"ob{t}")
                    if k == 0:
                        st['ps'] = pp_mm.tile([128, 512], F32, tag="mm",
                                              name=f"psp{t}_{cc}")
                    nc.tensor.matmul(st['ps'][:],
                                     ytr[k][:, t * 128:(t + 1) * 128],
                                     wps[k][:, csl], start=(k == 0),
                                     stop=(k == PAIRS - 1))
                    if k == PAIRS - 1:
                        if scalar_copy:
                            nc.scalar.copy(st['ob'][:, csl], st['ps'][:])
                        else:
                            nc.vector.tensor_copy(st['ob'][:, csl],
                                                  st['ps'][:])
                        if cc == 1:
                            nc.sync.dma_start(out=out_d.ap()
                                              [t * 128:(t + 1) * 128, :],
                                              in_=st['ob'][:])
                return [lambda cc=cc, k=k: emit(cc, k)
                        for cc in range(2) for k in range(PAIRS)]

            # ---- filler queue: (deadline, cost, op). deadline = hp*4+qb
            # index before which the op must have been emitted; 99 = end.
            # cost ~ matmul-equivalents the op puts on the PE.
            queue = []
            budget = [0.0]

            def enq(deadline, ops, cost=1):
                pos = len(queue)
                while pos > 0 and queue[pos - 1][0] > deadline:
                    pos -= 1
                queue[pos:pos] = [(deadline, cost, op) for op in ops]

            def pop_fillers(allowance):
                budget[0] += allowance
                while queue and budget[0] >= queue[0][1]:
                    _, cost, op = queue.pop(0)
                    op()
                    budget[0] -= cost

            def drain(deadline):
                while queue and queue[0][0] <= deadline:
                    _, _, op = queue.pop(0)
                    op()
                budget[0] = 0.0

            # upfront: QK pair0 (all chunks) + V tiles 0-7
            for n in range(NQ):
                for op in qk_chunk_ops(0, n):
                    op()
            for t in range(8):
                for op in v_tile_ops(t):
                    op()

            # queued work (deadline order)
            for t in range(8, 12):
                enq(2, v_tile_ops(t))
            for t in range(12, 16):
                enq(3, v_tile_ops(t))
            for n in range(NQ):
                enq(4 + n, qk_chunk_ops(1, n))
            for n in range(NQ):
                enq(8 + n, qk_chunk_ops(2, n))
            for n in range(NQ):
                enq(12 + n, qk_chunk_ops(3, n))

            # ---- attention ----
            for hp in range(PAIRS):
                qt, kt = qkt[hp]
                ynat[hp] = p_yn.tile([128, T], BF16, tag="yn",
                                     name=f"yn{hp}")
                ytr[hp] = p_yt.tile([128, T], BF16, tag="ytr",
                                    name=f"ytr{hp}")
                for qb in range(NQ):
                    drain(hp * 4 + qb)
                    # transposes become available with one-qb lag
                    if qb >= 1:
                        enq(hp * 4 + qb + 1, t_ops(hp, qb - 1))
                    elif hp >= 1:
                        enq(hp * 4 + 1, t_ops(hp - 1, 3))
                    if hp == PAIRS - 1 and qb >= 1:
                        # proj for t-tiles whose ytr just became available
                        for t in range((qb - 1) * 4, qb * 4):
                            enq(99, proj_ops(t))

                    yaccs = [pp_y.tile([128, 512], F32, tag="ya",
                                       name=f"ya{hp}_{qb}_{i}")
                             for i in range(2)]
                    ntk = 4 * qb + 4

                    def s_pair(tk):
                        j = tk - 4 * qb
                        q0 = 128 * j if j > 0 else 0
                        st_ = pp_st.tile([128, 1024], F32, tag="st",
                                         name=f"st{hp}_{qb}_{tk}")
                        qsl = slice(qb * 512 + q0, (qb + 1) * 512)
                        for h in range(2):
                            prt = slice(64 * h, 64 * h + 64)
                            nc.tensor.matmul(
                                st_[:, 512 * h + q0:512 * h + 512],
                                kt[prt, tk * 128:(tk + 1) * 128],
                                qt[prt, qsl], start=True, stop=True)
                        return st_, q0

                    def pv_tile(tk, px):
                        j = tk - 4 * qb
                        for qs in range(max(0, j), 4):
                            for h in range(2):
                                lv = hp * 2 + h
                                acc = yaccs[qs // 2]
                                off = (qs % 2) * 256 + h * 128
                                nc.tensor.matmul(
                                    acc[:, off:off + 65],
                                    px[:, h * 512 + qs * 128:
                                       h * 512 + (qs + 1) * 128],
                                    vaugs[tk][:, lv * 65:lv * 65 + 65],
                                    start=(tk == 0 and h == 0
                                           and qs % 2 == 0),
                                    stop=(tk == 4 * qb + qs))

                    st_cur, q0_cur = s_pair(0)
                    for tk in range(ntk):
                        j = tk - 4 * qb
                        q0 = q0_cur
                        st_ = st_cur
                        px = p_px.tile([128, 1024], BF16, tag="px")
                        nc.scalar.activation(
                            px[:].rearrange("p (r f) -> p r f", r=2)
                            [:, :, q0:512],
                            st_[:].rearrange("p (r f) -> p r f", r=2)
                            [:, :, q0:512],
                            EXP, scale=0.125)
                        if tk + 1 < ntk:
                            st_cur, q0_cur = s_pair(tk + 1)
                        pop_fillers(2)
                        if j >= 0:
                            pxm = (px[:].rearrange("p (r f) -> p r f", r=2)
                                   [:, :, q0:q0 + 128])
                            nc.gpsimd.tensor_mul(
                                pxm, pxm,
                                tri[:].unsqueeze(1)
                                .broadcast_to([128, 2, 128]))
                        pv_tile(tk, px)
                    # normalize: stage each yacc bank to SBUF, then GPSIMD
                    # divides by the per-partition denominator (col 64).
                    for bank in range(2):
                        stg = p_stg.tile([128, 512], F32, tag="stg",
                                         name=f"stg{hp}_{qb}_{bank}")
                        nc.vector.tensor_copy(stg[:], yaccs[bank][:])
                        for half in range(2):
                            for h in range(2):
                                qs = bank * 2 + half
                                off = half * 256 + h * 128
                                nc.gpsimd.normalize_recip(
                                    ynat[hp][:, (qb * 4 + qs) * 128
                                             + h * 64:
                                             (qb * 4 + qs) * 128
                                             + h * 64 + 64],
                                    stg[:, off:off + 64],
                                    stg[:, off + 64:off + 65])

            # tail: leftover queued proj first (keeps PE busy while the
            # last qb's normalize drains), then final transposes
            drain(99)
            for op in t_ops(PAIRS - 1, 3):
                op()
            for t in range(8, TT):
                for op in proj_ops(t, scalar_copy=True):
                    op()
            drain(100)
    nc.compile()
    return nc


def _get_nc(bias=False):
    key = ("nc", bias)
    if key not in _NC_CACHE:
        _NC_CACHE[key] = _build(bias=bias)
    return _NC_CACHE[key]


def kernel(x, w_attn, b_attn, w_proj, b_proj):
    x = np.asarray(x, dtype=np.float32)
    w_attn = np.asarray(w_attn, dtype=np.float32)
    b_attn = np.asarray(b_attn, dtype=np.float32)
    w_proj = np.asarray(w_proj, dtype=np.float32)
    b_proj = np.asarray(b_proj, dtype=np.float32)
    nc = _get_nc(bias=bool(np.any(b_attn)))
    bf = ml_dtypes.bfloat16
    in_maps = []
    for c in range(8):
        b, g = divmod(c, 2)
        xT = np.ascontiguousarray(x[b].T).astype(bf)
        s = 512 * g
        wqkv = np.ascontiguousarray(np.concatenate(
            [w_attn[:, s:s + 512],
             w_attn[:, 1024 + s:1024 + s + 512],
             w_attn[:, 2048 + s:2048 + s + 512]], axis=1)).astype(bf)
        bqkv = np.ascontiguousarray(np.concatenate(
            [b_attn[s:s + 512], b_attn[1024 + s:1024 + s + 512],
             b_attn[2048 + s:2048 + s + 512]])).astype(bf)
        wp = np.ascontiguousarray(w_proj[s:s + 512, :]).astype(bf)
        in_maps.append({"xT": xT, "wqkv": wqkv, "bqkv": bqkv, "wp": wp})
    globals()["_last_in_maps"] = in_maps
    res = run_bass_kernel_spmd(nc, in_maps, list(range(8)))
    out = np.empty((B, T, C), dtype=np.float32)
    for b in range(B):
        out[b] = res.results[2 * b]["out"] + res.results[2 * b + 1]["out"]
    out += b_proj
    return out


# revision 24
# speedup vs baseline: 1.7576x; 1.7576x over previous
"""Causal self-attention (B=4, T=2048, C=1024, 16 heads) on 8 NeuronCores.

Sharding: core c -> batch b=c//2, head group g=c%2 (8 heads each).
Each core computes q,k,v for its 8 heads, causal attention, and a partial
output projection (row-slice of w_proj). Host sums the two partials per
batch and adds b_proj.

v3 design:
  - all matmul operands bf16 (1 cycle/row on PE; fp32r ran fp32_mode=HIGH
    at 2-4 cycles/row).
  - S^T tiles [k,q] via row-tiled head pairs (K=64 stationaries at base
    partitions 0/64 run concurrently on the two PE row halves).
  - triangular restriction: diagonal k-tiles only compute q >= k; causal
    mask multiply only on the leading 128-wide triangle block.
  - PV in natural layout: Y[q,hd] = px_slice^T @ v_aug, px [k,128q] as
    stationary, v plus a ones column (-> softmax denominator at col 64)
    as the 65-wide moving operand. 4 (qs,h) accumulators share a PSUM
    bank; only the bank's first matmul uses start=True (start clears
    has_written for the WHOLE bank).
  - normalization: DVE copies each yacc bank to SBUF, then GPSIMD
    normalize_recip does y/denom per partition (keeps the DVE free for
    mask multiplies).
  - y_nat transposed back per 128-tile on the PE for the projection.
  - software pipelining via a global deadline-tagged filler queue: V
    tiles, QK projection chunks, y transposes and projection groups are
    popped 2 per tk into the PE's exp-wait bubbles; forced drains happen
    only when the next consumer needs them.
  - exp(tk) emitted before S(tk+1) so ACT is never gated by PV/fillers.
  - warmup matmuls keep the PE HAM unthrottled during the initial DMA.
"""
import numpy as np
import ml_dtypes
import concourse.bass as bass
from concourse import bacc
import concourse.tile as tile
import concourse.mybir as mybir
from concourse.bass_utils import run_bass_kernel_spmd

B, T, C = 4, 2048, 1024
HD = 64            # head dim
PAIRS = 4          # local head pairs (8 heads per core)
KT = C // 128      # 8 contraction tiles for qkv
TT = T // 128      # 16 row tiles of T
NQ = T // 512      # 4 query chunks of 512
BF16 = mybir.dt.bfloat16
F32 = mybir.dt.float32
EXP = mybir.ActivationFunctionType.Exp
WARMUP_MM = 90

_NC_CACHE = {}


def _build(bias=False):
    nc = bacc.Bacc("TRN2", target_bir_lowering=False, debug=False)
    xT_d = nc.dram_tensor("xT", [C, T], BF16, kind="ExternalInput")
    wqkv_d = nc.dram_tensor("wqkv", [C, 1536], BF16, kind="ExternalInput")
    bqkv_d = nc.dram_tensor("bqkv", [1536], BF16, kind="ExternalInput")
    wp_d = nc.dram_tensor("wp", [512, C], BF16, kind="ExternalInput")
    out_d = nc.dram_tensor("out", [T, C], F32, kind="ExternalOutput")

    tri_np = np.zeros((128, 128), dtype=np.float32)
    for p in range(128):
        tri_np[p, p:] = 1.0
    tri_d = nc.inline_tensor(tri_np.astype(ml_dtypes.bfloat16), name="tri")
    eye_d = nc.inline_tensor(np.eye(128, dtype=np.float32)
                             .astype(ml_dtypes.bfloat16), name="eye")
    ones_d = nc.inline_tensor(np.ones((1, 128), dtype=np.float32)
                              .astype(ml_dtypes.bfloat16), name="onesr")

    with tile.TileContext(nc) as tc:
        with (
            tc.tile_pool(name="x", bufs=8) as p_x,
            tc.tile_pool(name="w", bufs=8) as p_w,
            tc.tile_pool(name="wp", bufs=4) as p_wp,
            tc.tile_pool(name="va", bufs=16) as p_va,
            tc.tile_pool(name="qk", bufs=4) as p_qk,
            tc.tile_pool(name="px", bufs=5) as p_px,
            tc.tile_pool(name="stg", bufs=4) as p_stg,
            tc.tile_pool(name="yn", bufs=4) as p_yn,
            tc.tile_pool(name="yt", bufs=4) as p_yt,
            tc.tile_pool(name="ob", bufs=2) as p_ob,
            tc.tile_pool(name="tiny", bufs=2) as p_tiny,
            tc.tile_pool(name="st", bufs=2, space="PSUM") as pp_st,
            tc.tile_pool(name="ya", bufs=2, space="PSUM") as pp_y,
            tc.tile_pool(name="mm", bufs=2, space="PSUM") as pp_mm,
        ):
            # ---- constants ----
            tri = p_tiny.tile([128, 128], BF16, tag="tri")
            nc.sync.dma_start(out=tri[:], in_=tri_d.ap())
            eye = p_tiny.tile([128, 128], BF16, tag="eye")
            nc.sync.dma_start(out=eye[:], in_=eye_d.ap())
            wdum = p_tiny.tile([128, 512], BF16, tag="wdum")
            nc.vector.memset(wdum[:], 0.0)

            # warmup: keep PE busy (HAM unthrottled) during input DMA
            warm = pp_st.tile([128, 1024], F32, tag="st", name="warm")
            for i in range(WARMUP_MM):
                nc.tensor.matmul(warm[:, 0:512], eye[:], wdum[:],
                                 start=True, stop=True)

            # ---- input loads ----
            xts = []
            for k in range(KT):
                t_ = p_x.tile([128, T], BF16, tag="xt", name=f"xt{k}")
                nc.sync.dma_start(out=t_[:],
                                  in_=xT_d.ap()[k * 128:(k + 1) * 128, :])
                xts.append(t_)
            wqs = []
            for k in range(KT):
                w = p_w.tile([128, 1536], BF16, tag="wq", name=f"wq{k}")
                nc.sync.dma_start(out=w[:],
                                  in_=wqkv_d.ap()[k * 128:(k + 1) * 128, :])
                wqs.append(w)
            wps = []
            for k in range(PAIRS):
                w = p_wp.tile([128, C], BF16, tag="wp", name=f"wp{k}")
                nc.sync.dma_start(out=w[:],
                                  in_=wp_d.ap()[k * 128:(k + 1) * 128, :])
                wps.append(w)
            if bias:
                ones_r = p_tiny.tile([1, 128], BF16, tag="onesr")
                nc.sync.dma_start(out=ones_r[:], in_=ones_d.ap())
                bv = p_tiny.tile([1, 512], BF16, tag="bv")
                nc.sync.dma_start(out=bv[:],
                                  in_=bqkv_d.ap()[1024:1536].unsqueeze(0))
                bqs, bks = [], []
                for hp in range(PAIRS):
                    bq = p_tiny.tile([128, 1], BF16, tag=f"bq{hp}")
                    nc.sync.dma_start(out=bq[:], in_=bqkv_d.ap()
                                      [hp * 128:(hp + 1) * 128].unsqueeze(1))
                    bqs.append(bq)
                    bk = p_tiny.tile([128, 1], BF16, tag=f"bk{hp}")
                    nc.sync.dma_start(out=bk[:], in_=bqkv_d.ap()
                                      [512 + hp * 128:512 + (hp + 1) * 128]
                                      .unsqueeze(1))
                    bks.append(bk)

            # ---- emit helpers ----
            vaugs = [None] * TT

            def v_tile_ops(t):
                """9 closures producing vaug[t]."""
                st = {}

                def mm(k):
                    if k == 0:
                        st['ps'] = pp_mm.tile([128, 512], F32, tag="mm",
                                              name=f"psv{t}")
                    nc.tensor.matmul(st['ps'][:],
                                     xts[k][:, t * 128:(t + 1) * 128],
                                     wqs[k][:, 1024:1536], start=(k == 0),
                                     stop=(k == KT - 1 and not bias))
                    if k == KT - 1:
                        if bias:
                            nc.tensor.matmul(st['ps'][:], ones_r[:], bv[:],
                                             start=False, stop=True)
                        va = p_va.tile([128, 8 * 65], BF16, tag="va",
                                       name=f"va{t}")
                        nc.vector.tensor_copy(
                            va[:].rearrange("p (l c) -> p l c", c=65)
                            [:, :, 0:64],
                            st['ps'][:].rearrange("p (l c) -> p l c", c=64))
                        nc.gpsimd.memset(
                            va[:].rearrange("p (l c) -> p l c", c=65)
                            [:, :, 64:65], 1.0)
                        vaugs[t] = va
                return [lambda k=k: mm(k) for k in range(KT)]

            qkt = {}

            def qk_chunk_ops(hp, n):
                """16 closures: 8 q-matmuls then 8 k-matmuls for chunk n."""
                sl = slice(n * 512, (n + 1) * 512)
                st = {}

                def emit_q(k):
                    if n == 0 and k == 0:
                        qkt[hp] = (
                            p_qk.tile([128, T], BF16, tag="qk",
                                      name=f"qt{hp}"),
                            p_qk.tile([128, T], BF16, tag="qk",
                                      name=f"kt{hp}"),
                        )
                    if k == 0:
                        st['q'] = pp_mm.tile([128, 512], F32, tag="mm",
                                             name=f"psq{hp}_{n}")
                    nc.tensor.matmul(st['q'][:],
                                     wqs[k][:, hp * 128:(hp + 1) * 128],
                                     xts[k][:, sl], start=(k == 0),
                                     stop=(k == KT - 1))
                    if k == KT - 1:
                        if bias:
                            nc.vector.tensor_scalar_add(
                                qkt[hp][0][:, sl], st['q'][:],
                                bqs[hp][:, 0:1])
                        else:
                            nc.vector.tensor_copy(qkt[hp][0][:, sl],
                                                  st['q'][:])

                def emit_k(k):
                    if k == 0:
                        st['k'] = pp_mm.tile([128, 512], F32, tag="mm",
                                             name=f"psk{hp}_{n}")
                    nc.tensor.matmul(st['k'][:],
                                     wqs[k][:, 512 + hp * 128:
                                             512 + (hp + 1) * 128],
                                     xts[k][:, sl], start=(k == 0),
                                     stop=(k == KT - 1))
                    if k == KT - 1:
                        if bias:
                            nc.vector.tensor_scalar_add(
                                qkt[hp][1][:, sl], st['k'][:],
                                bks[hp][:, 0:1])
                        else:
                            nc.vector.tensor_copy(qkt[hp][1][:, sl],
                                                  st['k'][:])

                return ([lambda k=k: emit_q(k) for k in range(KT)]
                        + [lambda k=k: emit_k(k) for k in range(KT)])

            ynat, ytr = {}, {}

            def t_ops(hp, qb):
                """Transpose 4 t-tiles of pair hp's natural y for qb."""
                st = {}

                def emit(i):
                    if i == 0:
                        st['tp'] = pp_mm.tile([128, 512], BF16, tag="mm",
                                              name=f"tp{hp}_{qb}")
                    t = qb * 4 + i
                    nc.tensor.transpose(st['tp'][:, i * 128:(i + 1) * 128],
                                        ynat[hp][:, t * 128:(t + 1) * 128],
                                        eye[:])
                    nc.vector.tensor_copy(
                        ytr[hp][:, t * 128:(t + 1) * 128],
                        st['tp'][:, i * 128:(i + 1) * 128])
                return [lambda i=i: emit(i) for i in range(4)]

            def proj_ops(t, scalar_copy=False):
                """8 single-matmul closures for out t-tile + copy + DMA."""
                st = {}

                def emit(cc, k):
                    csl = slice(cc * 512, (cc + 1) * 512)
                    if cc == 0 and k == 0:
                        st['ob'] = p_ob.tile([128, 1024], F32, tag="ob",
                                             name=f"ob{t}")
                    if k == 0:
                        st['ps'] = pp_mm.tile([128, 512], F32, tag="mm",
                                              name=f"psp{t}_{cc}")
                    nc.tensor.matmul(st['ps'][:],
                                     ytr[k][:, t * 128:(t + 1) * 128],
                                     wps[k][:, csl], start=(k == 0),
                                     stop=(k == PAIRS - 1))
                    if k == PAIRS - 1:
                        if scalar_copy:
                            nc.scalar.copy(st['ob'][:, csl], st['ps'][:])
                        else:
                            nc.vector.tensor_copy(st['ob'][:, csl],
                                                  st['ps'][:])
                        if cc == 1:
                            nc.sync.dma_start(out=out_d.ap()
                                              [t * 128:(t + 1) * 128, :],
                                              in_=st['ob'][:])
                return [lambda cc=cc, k=k: emit(cc, k)
                        for cc in range(2) for k in range(PAIRS)]

            # ---- filler queue: (deadline, cost, op). deadline = hp*4+qb
            # index before which the op must have been emitted; 99 = end.
            # cost ~ matmul-equivalents the op puts on the PE.
            queue = []
            budget = [0.0]

            def enq(deadline, ops, cost=1):
                pos = len(queue)
                while pos > 0 and queue[pos - 1][0] > deadline:
                    pos -= 1
                queue[pos:pos] = [(deadline, cost, op) for op in ops]

            def pop_fillers(allowance):
                budget[0] += allowance
                while queue and budget[0] >= queue[0][1]:
                    _, cost, op = queue.pop(0)
                    op()
                    budget[0] -= cost

            def drain(deadline):
                while queue and queue[0][0] <= deadline:
                    _, _, op = queue.pop(0)
                    op()
                budget[0] = 0.0

            # upfront: QK pair0 (all chunks) + V tiles 0-7
            for n in range(NQ):
                for op in qk_chunk_ops(0, n):
                    op()
            for t in range(8):
                for op in v_tile_ops(t):
                    op()

            # queued work (deadline order)
            for t in range(8, 12):
                enq(2, v_tile_ops(t))
            for t in range(12, 16):
                enq(3, v_tile_ops(t))
            for n in range(NQ):
                enq(4 + n, qk_chunk_ops(1, n))
            for n in range(NQ):
                enq(8 + n, qk_chunk_ops(2, n))
            for n in range(NQ):
                enq(12 + n, qk_chunk_ops(3, n))

            # ---- attention ----
            for hp in range(PAIRS):
                qt, kt = qkt[hp]
                ynat[hp] = p_yn.tile([128, T], BF16, tag="yn",
                                     name=f"yn{hp}")
                ytr[hp] = p_yt.tile([128, T], BF16, tag="ytr",
                                    name=f"ytr{hp}")
                for qb in range(NQ):
                    drain(hp * 4 + qb)
                    # transposes become available with one-qb lag
                    if qb >= 1:
                        enq(hp * 4 + qb + 1, t_ops(hp, qb - 1))
                    elif hp >= 1:
                        enq(hp * 4 + 1, t_ops(hp - 1, 3))
                    if hp == PAIRS - 1 and qb >= 2:
                        # proj for t-tiles whose ytr completed (qb-2)
                        for t in range((qb - 2) * 4, (qb - 1) * 4):
                            enq(99, proj_ops(t))

                    yaccs = [pp_y.tile([128, 512], F32, tag="ya",
                                       name=f"ya{hp}_{qb}_{i}")
                             for i in range(2)]
                    ntk = 4 * qb + 4

                    def s_pair(tk):
                        j = tk - 4 * qb
                        q0 = 128 * j if j > 0 else 0
                        st_ = pp_st.tile([128, 1024], F32, tag="st",
                                         name=f"st{hp}_{qb}_{tk}")
                        qsl = slice(qb * 512 + q0, (qb + 1) * 512)
                        for h in range(2):
                            prt = slice(64 * h, 64 * h + 64)
                            nc.tensor.matmul(
                                st_[:, 512 * h + q0:512 * h + 512],
                                kt[prt, tk * 128:(tk + 1) * 128],
                                qt[prt, qsl], start=True, stop=True)
                        return st_, q0

                    def pv_tile(tk, px):
                        j = tk - 4 * qb
                        for qs in range(max(0, j), 4):
                            for h in range(2):
                                lv = hp * 2 + h
                                acc = yaccs[qs // 2]
                                off = (qs % 2) * 256 + h * 128
                                nc.tensor.matmul(
                                    acc[:, off:off + 65],
                                    px[:, h * 512 + qs * 128:
                                       h * 512 + (qs + 1) * 128],
                                    vaugs[tk][:, lv * 65:lv * 65 + 65],
                                    start=(tk == 0 and h == 0
                                           and qs % 2 == 0),
                                    stop=(tk == 4 * qb + qs))

                    st_cur, q0_cur = s_pair(0)
                    for tk in range(ntk):
                        j = tk - 4 * qb
                        q0 = q0_cur
                        st_ = st_cur
                        px = p_px.tile([128, 1024], BF16, tag="px")
                        nc.scalar.activation(
                            px[:].rearrange("p (r f) -> p r f", r=2)
                            [:, :, q0:512],
                            st_[:].rearrange("p (r f) -> p r f", r=2)
                            [:, :, q0:512],
                            EXP, scale=0.125)
                        if tk + 1 < ntk:
                            st_cur, q0_cur = s_pair(tk + 1)
                        pop_fillers(2)
                        if j >= 0:
                            pxm = (px[:].rearrange("p (r f) -> p r f", r=2)
                                   [:, :, q0:q0 + 128])
                            nc.vector.tensor_mul(
                                pxm, pxm,
                                tri[:].unsqueeze(1)
                                .broadcast_to([128, 2, 128]))
                        pv_tile(tk, px)
                    # normalize: stage each yacc bank to SBUF, then GPSIMD
                    # divides by the per-partition denominator (col 64).
                    for bank in range(2):
                        stg = p_stg.tile([128, 512], F32, tag="stg",
                                         name=f"stg{hp}_{qb}_{bank}")
                        nc.vector.tensor_copy(stg[:], yaccs[bank][:])
                        for half in range(2):
                            for h in range(2):
                                qs = bank * 2 + half
                                off = half * 256 + h * 128
                                nc.gpsimd.normalize_recip(
                                    ynat[hp][:, (qb * 4 + qs) * 128
                                             + h * 64:
                                             (qb * 4 + qs) * 128
                                             + h * 64 + 64],
                                    stg[:, off:off + 64],
                                    stg[:, off + 64:off + 65])

            # tail: leftover queued proj first (keeps PE busy while the
            # last qb's normalize drains), then final transposes
            drain(99)
            for op in t_ops(PAIRS - 1, 3):
                op()
            for t in range(8, TT):
                for op in proj_ops(t, scalar_copy=True):
                    op()
            drain(100)
    nc.compile()
    return nc


def _get_nc(bias=False):
    key = ("nc", bias)
    if key not in _NC_CACHE:
        _NC_CACHE[key] = _build(bias=bias)
    return _NC_CACHE[key]


def kernel(x, w_attn, b_attn, w_proj, b_proj):
    x = np.asarray(x, dtype=np.float32)
    w_attn = np.asarray(w_attn, dtype=np.float32)
    b_attn = np.asarray(b_attn, dtype=np.float32)
    w_proj = np.asarray(w_proj, dtype=np.float32)
    b_proj = np.asarray(b_proj, dtype=np.float32)
    nc = _get_nc(bias=bool(np.any(b_attn)))
    bf = ml_dtypes.bfloat16
    in_maps = []
    for c in range(8):
        b, g = divmod(c, 2)
        xT = np.ascontiguousarray(x[b].T).astype(bf)
        s = 512 * g
        wqkv = np.ascontiguousarray(np.concatenate(
            [w_attn[:, s:s + 512],
             w_attn[:, 1024 + s:1024 + s + 512],
             w_attn[:, 2048 + s:2048 + s + 512]], axis=1)).astype(bf)
        bqkv = np.ascontiguousarray(np.concatenate(
            [b_attn[s:s + 512], b_attn[1024 + s:1024 + s + 512],
             b_attn[2048 + s:2048 + s + 512]])).astype(bf)
        wp = np.ascontiguousarray(w_proj[s:s + 512, :]).astype(bf)
        in_maps.append({"xT": xT, "wqkv": wqkv, "bqkv": bqkv, "wp": wp})
    globals()["_last_in_maps"] = in_maps
    res = run_bass_kernel_spmd(nc, in_maps, list(range(8)))
    out = np.empty((B, T, C), dtype=np.float32)
    for b in range(B):
        out[b] = res.results[2 * b]["out"] + res.results[2 * b + 1]["out"]
    out += b_proj
    return out


# revision 25
# speedup vs baseline: 1.8183x; 1.0346x over previous
"""Causal self-attention (B=4, T=2048, C=1024, 16 heads) on 8 NeuronCores.

Sharding: core c -> batch b=c//2, head group g=c%2 (8 heads each).
Each core computes q,k,v for its 8 heads, causal attention, and a partial
output projection (row-slice of w_proj). Host sums the two partials per
batch and adds b_proj.

v3 design:
  - all matmul operands bf16 (1 cycle/row on PE; fp32r ran fp32_mode=HIGH
    at 2-4 cycles/row).
  - S^T tiles [k,q] via row-tiled head pairs (K=64 stationaries at base
    partitions 0/64 run concurrently on the two PE row halves).
  - triangular restriction: diagonal k-tiles only compute q >= k; causal
    mask multiply only on the leading 128-wide triangle block.
  - PV in natural layout: Y[q,hd] = px_slice^T @ v_aug, px [k,128q] as
    stationary, v plus a ones column (-> softmax denominator at col 64)
    as the 65-wide moving operand. 4 (qs,h) accumulators share a PSUM
    bank; only the bank's first matmul uses start=True (start clears
    has_written for the WHOLE bank).
  - normalization: DVE copies each yacc bank to SBUF, then GPSIMD
    normalize_recip does y/denom per partition (keeps the DVE free for
    mask multiplies).
  - y_nat transposed back per 128-tile on the PE for the projection.
  - software pipelining via a global deadline-tagged filler queue: V
    tiles, QK projection chunks, y transposes and projection groups are
    popped 2 per tk into the PE's exp-wait bubbles; forced drains happen
    only when the next consumer needs them.
  - exp(tk) emitted before S(tk+1) so ACT is never gated by PV/fillers.
  - warmup matmuls keep the PE HAM unthrottled during the initial DMA.
"""
import numpy as np
import ml_dtypes
import concourse.bass as bass
from concourse import bacc
import concourse.tile as tile
import concourse.mybir as mybir
from concourse.bass_utils import run_bass_kernel_spmd

B, T, C = 4, 2048, 1024
HD = 64            # head dim
PAIRS = 4          # local head pairs (8 heads per core)
KT = C // 128      # 8 contraction tiles for qkv
TT = T // 128      # 16 row tiles of T
NQ = T // 512      # 4 query chunks of 512
BF16 = mybir.dt.bfloat16
F32 = mybir.dt.float32
EXP = mybir.ActivationFunctionType.Exp
WARMUP_MM = 90

_NC_CACHE = {}


def _build(bias=False):
    nc = bacc.Bacc("TRN2", target_bir_lowering=False, debug=False)
    xT_d = nc.dram_tensor("xT", [C, T], BF16, kind="ExternalInput")
    wqkv_d = nc.dram_tensor("wqkv", [C, 1536], BF16, kind="ExternalInput")
    bqkv_d = nc.dram_tensor("bqkv", [1536], BF16, kind="ExternalInput")
    wp_d = nc.dram_tensor("wp", [512, C], BF16, kind="ExternalInput")
    out_d = nc.dram_tensor("out", [T, C], F32, kind="ExternalOutput")

    tri_np = np.zeros((128, 128), dtype=np.float32)
    for p in range(128):
        tri_np[p, p:] = 1.0
    tri_d = nc.inline_tensor(tri_np.astype(ml_dtypes.bfloat16), name="tri")
    eye_d = nc.inline_tensor(np.eye(128, dtype=np.float32)
                             .astype(ml_dtypes.bfloat16), name="eye")
    ones_d = nc.inline_tensor(np.ones((1, 128), dtype=np.float32)
                              .astype(ml_dtypes.bfloat16), name="onesr")

    with tile.TileContext(nc) as tc:
        with (
            tc.tile_pool(name="x", bufs=8) as p_x,
            tc.tile_pool(name="w", bufs=8) as p_w,
            tc.tile_pool(name="wp", bufs=4) as p_wp,
            tc.tile_pool(name="va", bufs=16) as p_va,
            tc.tile_pool(name="qk", bufs=4) as p_qk,
            tc.tile_pool(name="px", bufs=5) as p_px,
            tc.tile_pool(name="stg", bufs=4) as p_stg,
            tc.tile_pool(name="yn", bufs=4) as p_yn,
            tc.tile_pool(name="yt", bufs=4) as p_yt,
            tc.tile_pool(name="ob", bufs=2) as p_ob,
            tc.tile_pool(name="tiny", bufs=2) as p_tiny,
            tc.tile_pool(name="st", bufs=2, space="PSUM") as pp_st,
            tc.tile_pool(name="ya", bufs=2, space="PSUM") as pp_y,
            tc.tile_pool(name="mm", bufs=2, space="PSUM") as pp_mm,
        ):
            # ---- constants ----
            tri = p_tiny.tile([128, 128], BF16, tag="tri")
            nc.sync.dma_start(out=tri[:], in_=tri_d.ap())
            eye = p_tiny.tile([128, 128], BF16, tag="eye")
            nc.sync.dma_start(out=eye[:], in_=eye_d.ap())
            wdum = p_tiny.tile([128, 512], BF16, tag="wdum")
            nc.vector.memset(wdum[:], 0.0)

            # warmup: keep PE busy (HAM unthrottled) during input DMA
            warm = pp_st.tile([128, 1024], F32, tag="st", name="warm")
            for i in range(WARMUP_MM):
                nc.tensor.matmul(warm[:, 0:512], eye[:], wdum[:],
                                 start=True, stop=True)

            # ---- input loads ----
            xts = []
            for k in range(KT):
                t_ = p_x.tile([128, T], BF16, tag="xt", name=f"xt{k}")
                nc.sync.dma_start(out=t_[:],
                                  in_=xT_d.ap()[k * 128:(k + 1) * 128, :])
                xts.append(t_)
            wqs = []
            for k in range(KT):
                w = p_w.tile([128, 1536], BF16, tag="wq", name=f"wq{k}")
                nc.sync.dma_start(out=w[:],
                                  in_=wqkv_d.ap()[k * 128:(k + 1) * 128, :])
                wqs.append(w)
            wps = []
            for k in range(PAIRS):
                w = p_wp.tile([128, C], BF16, tag="wp", name=f"wp{k}")
                nc.sync.dma_start(out=w[:],
                                  in_=wp_d.ap()[k * 128:(k + 1) * 128, :])
                wps.append(w)
            if bias:
                ones_r = p_tiny.tile([1, 128], BF16, tag="onesr")
                nc.sync.dma_start(out=ones_r[:], in_=ones_d.ap())
                bv = p_tiny.tile([1, 512], BF16, tag="bv")
                nc.sync.dma_start(out=bv[:],
                                  in_=bqkv_d.ap()[1024:1536].unsqueeze(0))
                bqs, bks = [], []
                for hp in range(PAIRS):
                    bq = p_tiny.tile([128, 1], BF16, tag=f"bq{hp}")
                    nc.sync.dma_start(out=bq[:], in_=bqkv_d.ap()
                                      [hp * 128:(hp + 1) * 128].unsqueeze(1))
                    bqs.append(bq)
                    bk = p_tiny.tile([128, 1], BF16, tag=f"bk{hp}")
                    nc.sync.dma_start(out=bk[:], in_=bqkv_d.ap()
                                      [512 + hp * 128:512 + (hp + 1) * 128]
                                      .unsqueeze(1))
                    bks.append(bk)

            # ---- emit helpers ----
            vaugs = [None] * TT

            def v_tile_ops(t):
                """9 closures producing vaug[t]."""
                st = {}

                def mm(k):
                    if k == 0:
                        st['ps'] = pp_mm.tile([128, 512], F32, tag="mm",
                                              name=f"psv{t}")
                    nc.tensor.matmul(st['ps'][:],
                                     xts[k][:, t * 128:(t + 1) * 128],
                                     wqs[k][:, 1024:1536], start=(k == 0),
                                     stop=(k == KT - 1 and not bias))
                    if k == KT - 1:
                        if bias:
                            nc.tensor.matmul(st['ps'][:], ones_r[:], bv[:],
                                             start=False, stop=True)
                        va = p_va.tile([128, 8 * 65], BF16, tag="va",
                                       name=f"va{t}")
                        nc.vector.tensor_copy(
                            va[:].rearrange("p (l c) -> p l c", c=65)
                            [:, :, 0:64],
                            st['ps'][:].rearrange("p (l c) -> p l c", c=64))
                        nc.gpsimd.memset(
                            va[:].rearrange("p (l c) -> p l c", c=65)
                            [:, :, 64:65], 1.0)
                        vaugs[t] = va
                return [lambda k=k: mm(k) for k in range(KT)]

            qkt = {}

            def qk_chunk_ops(hp, n):
                """16 closures: 8 q-matmuls then 8 k-matmuls for chunk n."""
                sl = slice(n * 512, (n + 1) * 512)
                st = {}

                def emit_q(k):
                    if n == 0 and k == 0:
                        qkt[hp] = (
                            p_qk.tile([128, T], BF16, tag="qk",
                                      name=f"qt{hp}"),
                            p_qk.tile([128, T], BF16, tag="qk",
                                      name=f"kt{hp}"),
                        )
                    if k == 0:
                        st['q'] = pp_mm.tile([128, 512], F32, tag="mm",
                                             name=f"psq{hp}_{n}")
                    nc.tensor.matmul(st['q'][:],
                                     wqs[k][:, hp * 128:(hp + 1) * 128],
                                     xts[k][:, sl], start=(k == 0),
                                     stop=(k == KT - 1))
                    if k == KT - 1:
                        if bias:
                            nc.vector.tensor_scalar_add(
                                qkt[hp][0][:, sl], st['q'][:],
                                bqs[hp][:, 0:1])
                        else:
                            nc.vector.tensor_copy(qkt[hp][0][:, sl],
                                                  st['q'][:])

                def emit_k(k):
                    if k == 0:
                        st['k'] = pp_mm.tile([128, 512], F32, tag="mm",
                                             name=f"psk{hp}_{n}")
                    nc.tensor.matmul(st['k'][:],
                                     wqs[k][:, 512 + hp * 128:
                                             512 + (hp + 1) * 128],
                                     xts[k][:, sl], start=(k == 0),
                                     stop=(k == KT - 1))
                    if k == KT - 1:
                        if bias:
                            nc.vector.tensor_scalar_add(
                                qkt[hp][1][:, sl], st['k'][:],
                                bks[hp][:, 0:1])
                        else:
                            nc.vector.tensor_copy(qkt[hp][1][:, sl],
                                                  st['k'][:])

                return ([lambda k=k: emit_q(k) for k in range(KT)]
                        + [lambda k=k: emit_k(k) for k in range(KT)])

            ynat, ytr = {}, {}

            def t_ops(hp, qb):
                """Transpose 4 t-tiles of pair hp's natural y for qb."""
                st = {}

                def emit(i):
                    if i == 0:
                        st['tp'] = pp_mm.tile([128, 512], BF16, tag="mm",
                                              name=f"tp{hp}_{qb}")
                    t = qb * 4 + i
                    nc.tensor.transpose(st['tp'][:, i * 128:(i + 1) * 128],
                                        ynat[hp][:, t * 128:(t + 1) * 128],
                                        eye[:])
                    nc.vector.tensor_copy(
                        ytr[hp][:, t * 128:(t + 1) * 128],
                        st['tp'][:, i * 128:(i + 1) * 128])
                return [lambda i=i: emit(i) for i in range(4)]

            def proj_ops(t, scalar_copy=False):
                """8 single-matmul closures for out t-tile + copy + DMA."""
                st = {}

                def emit(cc, k):
                    csl = slice(cc * 512, (cc + 1) * 512)
                    if cc == 0 and k == 0:
                        st['ob'] = p_ob.tile([128, 1024], F32, tag="ob",
                                             name=f"ob{t}")
                    if k == 0:
                        st['ps'] = pp_mm.tile([128, 512], F32, tag="mm",
                                              name=f"psp{t}_{cc}")
                    nc.tensor.matmul(st['ps'][:],
                                     ytr[k][:, t * 128:(t + 1) * 128],
                                     wps[k][:, csl], start=(k == 0),
                                     stop=(k == PAIRS - 1))
                    if k == PAIRS - 1:
                        if scalar_copy:
                            nc.scalar.copy(st['ob'][:, csl], st['ps'][:])
                        else:
                            nc.vector.tensor_copy(st['ob'][:, csl],
                                                  st['ps'][:])
                        if cc == 1:
                            nc.sync.dma_start(out=out_d.ap()
                                              [t * 128:(t + 1) * 128, :],
                                              in_=st['ob'][:])
                return [lambda cc=cc, k=k: emit(cc, k)
                        for cc in range(2) for k in range(PAIRS)]

            # ---- filler queue: (deadline, cost, op). deadline = hp*4+qb
            # index before which the op must have been emitted; 99 = end.
            # cost ~ matmul-equivalents the op puts on the PE.
            queue = []
            budget = [0.0]

            def enq(deadline, ops, cost=1):
                pos = len(queue)
                while pos > 0 and queue[pos - 1][0] > deadline:
                    pos -= 1
                queue[pos:pos] = [(deadline, cost, op) for op in ops]

            def pop_fillers(allowance):
                budget[0] += allowance
                while queue and budget[0] >= queue[0][1]:
                    _, cost, op = queue.pop(0)
                    op()
                    budget[0] -= cost

            def drain(deadline):
                while queue and queue[0][0] <= deadline:
                    _, _, op = queue.pop(0)
                    op()
                budget[0] = 0.0

            # upfront: QK pair0 (all chunks) + V tiles 0-7
            for n in range(NQ):
                for op in qk_chunk_ops(0, n):
                    op()
            for t in range(8):
                for op in v_tile_ops(t):
                    op()

            # queued work (deadline order)
            for t in range(8, 12):
                enq(2, v_tile_ops(t))
            for t in range(12, 16):
                enq(3, v_tile_ops(t))
            for n in range(NQ):
                enq(4 + n, qk_chunk_ops(1, n))
            for n in range(NQ):
                enq(8 + n, qk_chunk_ops(2, n))
            for n in range(NQ):
                enq(12 + n, qk_chunk_ops(3, n))

            # ---- attention ----
            for hp in range(PAIRS):
                qt, kt = qkt[hp]
                ynat[hp] = p_yn.tile([128, T], BF16, tag="yn",
                                     name=f"yn{hp}")
                ytr[hp] = p_yt.tile([128, T], BF16, tag="ytr",
                                    name=f"ytr{hp}")
                for qb in range(NQ):
                    drain(hp * 4 + qb)
                    # transposes become available with one-qb lag
                    if qb >= 1:
                        enq(hp * 4 + qb + 1, t_ops(hp, qb - 1))
                    elif hp >= 1:
                        enq(hp * 4 + 1, t_ops(hp - 1, 3))
                    if hp == PAIRS - 1 and qb >= 1:
                        # proj for t-tiles whose ytr just became available
                        for t in range((qb - 1) * 4, qb * 4):
                            enq(99, proj_ops(t))

                    yaccs = [pp_y.tile([128, 512], F32, tag="ya",
                                       name=f"ya{hp}_{qb}_{i}")
                             for i in range(2)]
                    ntk = 4 * qb + 4

                    def s_pair(tk):
                        j = tk - 4 * qb
                        q0 = 128 * j if j > 0 else 0
                        st_ = pp_st.tile([128, 1024], F32, tag="st",
                                         name=f"st{hp}_{qb}_{tk}")
                        qsl = slice(qb * 512 + q0, (qb + 1) * 512)
                        for h in range(2):
                            prt = slice(64 * h, 64 * h + 64)
                            nc.tensor.matmul(
                                st_[:, 512 * h + q0:512 * h + 512],
                                kt[prt, tk * 128:(tk + 1) * 128],
                                qt[prt, qsl], start=True, stop=True)
                        return st_, q0

                    def pv_tile(tk, px):
                        j = tk - 4 * qb
                        for qs in range(max(0, j), 4):
                            for h in range(2):
                                lv = hp * 2 + h
                                acc = yaccs[qs // 2]
                                off = (qs % 2) * 256 + h * 128
                                nc.tensor.matmul(
                                    acc[:, off:off + 65],
                                    px[:, h * 512 + qs * 128:
                                       h * 512 + (qs + 1) * 128],
                                    vaugs[tk][:, lv * 65:lv * 65 + 65],
                                    start=(tk == 0 and h == 0
                                           and qs % 2 == 0),
                                    stop=(tk == 4 * qb + qs))

                    st_cur, q0_cur = s_pair(0)
                    for tk in range(ntk):
                        j = tk - 4 * qb
                        q0 = q0_cur
                        st_ = st_cur
                        px = p_px.tile([128, 1024], BF16, tag="px")
                        nc.scalar.activation(
                            px[:].rearrange("p (r f) -> p r f", r=2)
                            [:, :, q0:512],
                            st_[:].rearrange("p (r f) -> p r f", r=2)
                            [:, :, q0:512],
                            EXP, scale=0.125)
                        if tk + 1 < ntk:
                            st_cur, q0_cur = s_pair(tk + 1)
                        pop_fillers(3 if hp == PAIRS - 1 else 2)
                        if j >= 0:
                            pxm = (px[:].rearrange("p (r f) -> p r f", r=2)
                                   [:, :, q0:q0 + 128])
                            nc.vector.tensor_mul(
                                pxm, pxm,
                                tri[:].unsqueeze(1)
                                .broadcast_to([128, 2, 128]))
                        pv_tile(tk, px)
                    # normalize: stage each yacc bank to SBUF, then GPSIMD
                    # divides by the per-partition denominator (col 64).
                    for bank in range(2):
                        stg = p_stg.tile([128, 512], F32, tag="stg",
                                         name=f"stg{hp}_{qb}_{bank}")
                        nc.vector.tensor_copy(stg[:], yaccs[bank][:])
                        for half in range(2):
                            for h in range(2):
                                qs = bank * 2 + half
                                off = half * 256 + h * 128
                                nc.gpsimd.normalize_recip(
                                    ynat[hp][:, (qb * 4 + qs) * 128
                                             + h * 64:
                                             (qb * 4 + qs) * 128
                                             + h * 64 + 64],
                                    stg[:, off:off + 64],
                                    stg[:, off + 64:off + 65])

            # tail: leftover queued proj first (keeps PE busy while the
            # last qb's normalize drains), then final transposes
            drain(99)
            for op in t_ops(PAIRS - 1, 3):
                op()
            for t in range(12, TT):
                for op in proj_ops(t, scalar_copy=True):
                    op()
            drain(100)
    nc.compile()
    return nc


def _get_nc(bias=False):
    key = ("nc", bias)
    if key not in _NC_CACHE:
        _NC_CACHE[key] = _build(bias=bias)
    return _NC_CACHE[key]


def kernel(x, w_attn, b_attn, w_proj, b_proj):
    x = np.asarray(x, dtype=np.float32)
    w_attn = np.asarray(w_attn, dtype=np.float32)
    b_attn = np.asarray(b_attn, dtype=np.float32)
    w_proj = np.asarray(w_proj, dtype=np.float32)
    b_proj = np.asarray(b_proj, dtype=np.float32)
    nc = _get_nc(bias=bool(np.any(b_attn)))
    bf = ml_dtypes.bfloat16
    in_maps = []
    for c in range(8):
        b, g = divmod(c, 2)
        xT = np.ascontiguousarray(x[b].T).astype(bf)
        s = 512 * g
        wqkv = np.ascontiguousarray(np.concatenate(
            [w_attn[:, s:s + 512],
             w_attn[:, 1024 + s:1024 + s + 512],
             w_attn[:, 2048 + s:2048 + s + 512]], axis=1)).astype(bf)
        bqkv = np.ascontiguousarray(np.concatenate(
            [b_attn[s:s + 512], b_attn[1024 + s:1024 + s + 512],
             b_attn[2048 + s:2048 + s + 512]])).astype(bf)
        wp = np.ascontiguousarray(w_proj[s:s + 512, :]).astype(bf)
        in_maps.append({"xT": xT, "wqkv": wqkv, "bqkv": bqkv, "wp": wp})
    globals()["_last_in_maps"] = in_maps
    res = run_bass_kernel_spmd(nc, in_maps, list(range(8)))
    out = np.empty((B, T, C), dtype=np.float32)
    for b in range(B):
        out[b] = res.results[2 * b]["out"] + res.results[2 * b + 1]["out"]
    out += b_proj
    return out
